# revision 20
# baseline (speedup 1.0000x reference)
"""Trainium2 Bass kernel for AttentionM (dense transformer block).

Computes, for x [4, 2048, 1024] and q/k/v CSS-gated projections:
    q = (x@Wq+bq)*sigmoid(x@Wqc+bqc)   -> [B, Sp, 16 heads, 16]
    k = likewise                        -> [B, Sp, 16, 16]
    v = likewise (64-wide heads)        -> [B, Sp, 16, 64]
    ctx = softmax(q k^T / 8) v          -> [B, S, 1024]
with Sp = S+16 zero-padded rows (pad tokens participate via bias-only css).

Sharding over 8 NeuronCores: 4-way data parallel over batch x 2-way tensor
parallel over heads (8 heads per core). Each core gets x[b] (padded) and its
head-slice of the weights, computes ctx[b, :, hg*512:(hg+1)*512].

Per-core dataflow (all matmuls in float32r, 1 cycle/row):
  1. x -> SBUF token-major, PE-transpose to xT [128, 8, 2064] (D on partitions)
  2. qT/kT feature-major [128=8h*16, S*] via W-stationary matmuls;
     sigmoid gate fused via ACT + one DVE scalar_tensor_tensor.
  3. v token-major [tok, 8, 65] with a ones column appended per head
     (bias added via a K=1 matmul against a ones column). The sigmoid is
     computed as 1/(1+exp(-z)) so the v phase shares the ACT exp table with
     attention and can be interleaved into the first head's k-tile loop.
  4. Per (qr, h): scoresT [k_tile, q] = kh^T qh on PE; exp on ACT (scale=1/8,
     no max subtraction -- scores are provably in [-3, 3]); ctxT [65, q]
     accumulated over k tiles in PSUM. Row 64 is the softmax denominator
     (exp @ ones), so ctx = rows 0..63 / row 64 -- no separate reduction.
     The ctx matmuls are software-pipelined one k-tile behind the scores
     matmuls so the PE never blocks on the current exp.
  5. PE-transpose ctxT back to token-major, divide, DMA out.
"""

import sys

if "/opt/trn_rl_repo" not in sys.path:
    sys.path.insert(0, "/opt/trn_rl_repo")

import numpy as np

import concourse.bacc as bacc
import concourse.mybir as mybir
import concourse.tile as tile
from concourse.bass_utils import run_bass_kernel_spmd
from concourse.masks import make_identity

F32 = mybir.dt.float32
F32R = mybir.dt.float32r
AF = mybir.ActivationFunctionType
ALU = mybir.AluOpType

B = 4
S = 2048          # real sequence
PAD = 16
SP = S + PAD      # padded sequence (k extent)
D = 1024
DC = D // 128     # 8 contraction chunks
HL = 8            # heads per core
QL = 16           # q/k head dim
VL = 64           # v head dim
NKT = SP // 128 + 1          # 17 k tiles (16 full + 16-row tail)
QR = 1024                    # q range per psum accumulator
SCALE = 1.0 / 8.0            # 1/sqrt(64)


def _build(repeat=1):
    nc = bacc.Bacc("TRN2", target_bir_lowering=False, debug=False, num_devices=8)

    x_d = nc.dram_tensor("x", [SP, D], F32R, kind="ExternalInput").ap()
    idr_d = nc.dram_tensor("idr", [128, 128], F32R, kind="ExternalInput").ap()
    bdiag_d = nc.dram_tensor("bdiag", [128, 8], F32R, kind="ExternalInput").ap()
    wq_d = nc.dram_tensor("wq", [D, 128], F32R, kind="ExternalInput").ap()
    wqc_d = nc.dram_tensor("wqc", [D, 128], F32R, kind="ExternalInput").ap()
    wk_d = nc.dram_tensor("wk", [D, 128], F32R, kind="ExternalInput").ap()
    wkc_d = nc.dram_tensor("wkc", [D, 128], F32R, kind="ExternalInput").ap()
    wv_d = nc.dram_tensor("wv", [D, 512], F32R, kind="ExternalInput").ap()
    wvc_d = nc.dram_tensor("wvc", [D, 512], F32R, kind="ExternalInput").ap()
    bq_d = nc.dram_tensor("bq", [128], F32, kind="ExternalInput").ap()
    bqc_d = nc.dram_tensor("bqc", [128], F32, kind="ExternalInput").ap()
    bk_d = nc.dram_tensor("bk", [128], F32, kind="ExternalInput").ap()
    bkc_d = nc.dram_tensor("bkc", [128], F32, kind="ExternalInput").ap()
    bv_d = nc.dram_tensor("bv", [512], F32R, kind="ExternalInput").ap()
    bvc_d = nc.dram_tensor("bvc", [512], F32R, kind="ExternalInput").ap()
    y_d = nc.dram_tensor("y", [S, 512], F32, kind="ExternalOutput").ap()

    with tile.TileContext(nc) as tc:
        for _ in range(repeat):
            _emit(nc, tc, x_d, idr_d, bdiag_d, wq_d, wqc_d, wk_d, wkc_d, wv_d,
                  wvc_d, bq_d, bqc_d, bk_d, bkc_d, bv_d, bvc_d, y_d)
    nc.compile()
    return nc


def _emit(nc, tc, x_d, idr_d, bdiag_d, wq_d, wqc_d, wk_d, wkc_d, wv_d,
          wvc_d, bq_d, bqc_d, bk_d, bkc_d, bv_d, bvc_d, y_d):
    # ---------------- long-lived pools ----------------
    const = tc.alloc_tile_pool(name="const", bufs=1)
    proj = tc.alloc_tile_pool(name="proj", bufs=1)
    pp_mm = tc.alloc_tile_pool(name="pp_mm", bufs=2, space="PSUM")
    pp_tp = tc.alloc_tile_pool(name="pp_tp", bufs=2, space="PSUM")
    pp_acc = tc.alloc_tile_pool(name="pp_acc", bufs=1, space="PSUM")

    ph13 = tc.alloc_tile_pool(name="ph13", bufs=1)
    # xT split by column range so consumers start before all of x is transposed
    xT_parts = [ph13.tile([128, DC, 528 if i == 3 else 512], F32R, name=f"xT{i}")
                for i in range(4)]

    def xTs(d, c0, csz):
        part = min(c0 // 512, 3)
        lo = c0 - part * 512
        assert lo + csz <= (528 if part == 3 else 512)
        return xT_parts[part][:, d, lo:lo + csz]
    wpool = tc.alloc_tile_pool(name="wpool", bufs=1)
    ph1 = tc.alloc_tile_pool(name="ph1", bufs=3)

    # f32r identity first: the x transposes block on it
    idr = const.tile([128, 128], F32R, name="idr")
    nc.sync.dma_start(out=idr, in_=idr_d)
    ident = const.tile([128, 128], F32)
    make_identity(nc, ident)

    # x tiles: the transpose pipeline is the critical path at start
    xts = []
    for t in range(NKT):
        tsz = min(128, SP - t * 128)
        xt = ph1.tile([128, D], F32R, name="xload")
        nc.sync.dma_start(out=xt[0:tsz, :], in_=x_d[t * 128: t * 128 + tsz, :])
        xts.append((xt, tsz))

    def ppart(bias_d, dtype=F32):
        t = const.tile([128, 1], dtype, name=f"b_{bias_d.name}")
        nc.sync.dma_start(out=t, in_=bias_d.unsqueeze(-1))
        return t

    bq_sb = ppart(bq_d)
    bqc_sb = ppart(bqc_d)
    bk_sb = ppart(bk_d)
    bkc_sb = ppart(bkc_d)
    bv_row = const.tile([1, 512], F32R, name="bv_row")
    nc.sync.dma_start(out=bv_row, in_=bv_d.unsqueeze(0))
    bvc_row = const.tile([1, 512], F32R, name="bvc_row")
    nc.sync.dma_start(out=bvc_row, in_=bvc_d.unsqueeze(0))

    # weights prefetched after the (tiny) biases; the big v weights last --
    # they are not needed until the v phase and must not delay phase 2
    wq = wpool.tile([128, DC, 128], F32R, name="wq")
    wqc = wpool.tile([128, DC, 128], F32R, name="wqc")
    wk = wpool.tile([128, DC, 128], F32R, name="wk")
    wkc = wpool.tile([128, DC, 128], F32R, name="wkc")
    wv = wpool.tile([128, DC, 512], F32R, name="wv")
    wvc = wpool.tile([128, DC, 512], F32R, name="wvc")
    for w_sb, w_dd in ((wk, wk_d), (wkc, wkc_d), (wq, wq_d), (wqc, wqc_d)):
        nc.sync.dma_start(out=w_sb, in_=w_dd.rearrange("(a p) c -> p a c", p=128))
    for w_sb, w_dd in ((wv, wv_d), (wvc, wvc_d)):
        wr = w_dd.rearrange("(a p) c -> p a c", p=128)
        for d in range(DC):
            nc.sync.dma_start(out=w_sb[:, d, :], in_=wr[:, d, :])

    qT = proj.tile([128, S], F32R, name="qT")       # [8h*16, q]
    kT = proj.tile([128, SP], F32R, name="kT")      # [8h*16, k]
    vt = proj.tile([128, NKT, HL, VL + 1], F32R, name="vt")  # token-major v + ones

    # ones column [1, 128] in f32r for the K=1 bias matmuls
    ones_col = const.tile([1, 128], F32R, name="ones_col")
    nc.scalar.activation(out=ones_col, in_=idr[0:1, :], func=AF.Copy,
                         scale=0.0, bias=1.0)

    # ---------------- phase 1: build xT (f32r transposes) ----------------
    for t in range(NKT):
        xt, tsz = xts[t]
        for half in range(2):
            tp = pp_tp.tile([128, 512], F32R, name="tp")
            for jj in range(4):
                d = half * 4 + jj
                nc.tensor.transpose(
                    out=tp[:, jj * 128: jj * 128 + tsz],
                    in_=xt[0:tsz, d * 128:(d + 1) * 128],
                    identity=idr[0:tsz, 0:tsz],
                )
            part = min(t // 4, 3)
            lo = t * 128 - part * 512
            nc.vector.tensor_copy(
                out=xT_parts[part][:, half * 4:(half + 1) * 4, lo:lo + tsz],
                in_=tp.rearrange("p (b c) -> p b c", b=4)[:, :, 0:tsz],
            )
    ph1.release()

    # ---------------- phase 2: q/k projections (feature-major) ----------------
    # pre-attention phases rotate a third psum slot through the (idle)
    # attention accumulator pool for deeper pipelining
    _rr = [0]

    def mm_tile():
        _rr[0] += 1
        if _rr[0] % 3 == 0:
            return pp_acc.tile([128, QR], F32, name="acc")
        return pp_mm.tile([128, 1024], F32, name="mm")

    sig2 = tc.alloc_tile_pool(name="sig2", bufs=2)
    for wl, wcl, bl, bcl, dest, cols in (
        (wk, wkc, bk_sb, bkc_sb, kT, SP),
        (wq, wqc, bq_sb, bqc_sb, qT, S),
    ):
        for c0 in range(0, cols, 512):
            csz = min(512, cols - c0)
            ps = mm_tile()
            for d in range(DC):
                nc.tensor.matmul(ps[:, 0:csz], wl[:, d, :], xTs(d, c0, csz),
                                 start=(d == 0), stop=(d == DC - 1))
            for d in range(DC):
                nc.tensor.matmul(ps[:, 512:512 + csz], wcl[:, d, :],
                                 xTs(d, c0, csz),
                                 start=(d == 0), stop=(d == DC - 1))
            sig = sig2.tile([128, 512], F32, name="sig")
            nc.scalar.activation(out=sig[:, 0:csz], in_=ps[:, 512:512 + csz],
                                 func=AF.Sigmoid, bias=bcl)
            nc.vector.scalar_tensor_tensor(
                out=dest[:, c0:c0 + csz], in0=ps[:, 0:csz], scalar=bl,
                in1=sig[:, 0:csz], op0=ALU.add, op1=ALU.mult)
    sig2.release()

    # ---------------- phase 3: v projection (token-major, ones col) ----------
    sig3 = tc.alloc_tile_pool(name="sig3", bufs=2)

    def emit_v_tile(t):
        """v'[t] = [(lin+bv) * sigmoid(linc+bvc) | 1], token-major."""
        tsz = min(128, SP - t * 128)
        tc0 = t * 128
        ps = mm_tile()
        for d in range(DC):
            nc.tensor.matmul(ps[0:tsz, 0:512], xTs(d, tc0, tsz), wv[:, d, :],
                             start=(d == 0), stop=False)
        nc.tensor.matmul(ps[0:tsz, 0:512], ones_col[:, 0:tsz], bv_row,
                         start=False, stop=True)
        for d in range(DC):
            nc.tensor.matmul(ps[0:tsz, 512:1024], xTs(d, tc0, tsz),
                             wvc[:, d, :], start=(d == 0), stop=False)
        nc.tensor.matmul(ps[0:tsz, 512:1024], ones_col[:, 0:tsz], bvc_row,
                         start=False, stop=True)
        sg = sig3.tile([128, 512], F32, name="sigv")
        nc.scalar.activation(out=sg[0:tsz, :], in_=ps[0:tsz, 512:1024],
                             func=AF.Sigmoid)
        nc.vector.tensor_tensor(
            out=vt[0:tsz, t, :, 0:VL],
            in0=ps[0:tsz, 0:512].rearrange("p (h v) -> p h v", h=HL),
            in1=sg[0:tsz, :].rearrange("p (h v) -> p h v", h=HL),
            op=ALU.mult)
        nc.scalar.activation(out=vt[:, t, :, VL:VL + 1],
                             in_=idr[:, 0:HL].unsqueeze(-1),
                             func=AF.Copy, scale=0.0, bias=1.0)

    for t in range(NKT - 1):
        emit_v_tile(t)
    # pad rows of x are zero, so v'_pad = bv * sigmoid(bvc) -- no matmul needed.
    # All 16 pad rows are identical; only row 0 is kept (used as a K=1 lhsT).
    sgp = sig3.tile([128, 512], F32, name="sigv")
    nc.scalar.activation(out=sgp[0:1, :], in_=bvc_row.bitcast(F32), func=AF.Sigmoid)
    nc.vector.tensor_tensor(
        out=vt[0:1, NKT - 1, :, 0:VL],
        in0=bv_row.bitcast(F32).rearrange("p (h v) -> p h v", h=HL),
        in1=sgp[0:1, :].rearrange("p (h v) -> p h v", h=HL),
        op=ALU.mult)
    nc.scalar.activation(out=vt[0:1, NKT - 1, :, VL:VL + 1],
                         in_=idr[0:1, 0:HL].unsqueeze(-1),
                         func=AF.Copy, scale=0.0, bias=1.0)
    sig3.release()
    ph1_released = True
    wpool.release()
    ph13.release()

    # ---------------- phase 4: attention ----------------
    stage = tc.alloc_tile_pool(name="stage", bufs=2)
    expp = tc.alloc_tile_pool(name="expp", bufs=4)
    ctp = tc.alloc_tile_pool(name="ctp", bufs=2)
    outp = tc.alloc_tile_pool(name="outp", bufs=2)
    rcp = tc.alloc_tile_pool(name="rcp", bufs=2)
    padp = tc.alloc_tile_pool(name="padp", bufs=1)

    # The 16 identical zero-pad k rows collapse into one rank-1 update:
    # acc += exp(s_pad/8 + ln 16) * v'_pad. Build block-diag pad-k [128, 8]
    # (head h's pad-k vector at rows 16h..16h+16), batch all heads' pad
    # scores into two [8, 1024] matmuls + one exp each, then restage to
    # partition 0 for the K=1 ctx update.
    padk = padp.tile([128, HL], F32R, name="padk")
    bdiag = padp.tile([128, HL], F32R, name="bdiag")
    nc.sync.dma_start(out=bdiag, in_=bdiag_d)
    nc.vector.tensor_scalar(out=padk, in0=bdiag,
                            scalar1=kT[:, S:S + 1].bitcast(F32),
                            scalar2=None, op0=ALU.mult)
    e_pad = padp.tile([HL, 2, QR], F32R, name="e_pad")
    LN16 = float(np.log(16.0))
    ln16_sb = padp.tile([128, 1], F32, name="ln16")
    nc.scalar.activation(out=ln16_sb, in_=idr[:, 0:1], func=AF.Copy,
                         scale=0.0, bias=LN16)
    for r in range(2):
        pps = pp_mm.tile([128, QR], F32, name="mm")
        for j in range(QR // 512):
            nc.tensor.matmul(pps[0:HL, j * 512:(j + 1) * 512], padk,
                             qT[:, r * QR + j * 512: r * QR + (j + 1) * 512],
                             start=True, stop=True)
        nc.scalar.activation(out=e_pad[:, r, :], in_=pps[0:HL, :],
                             func=AF.Exp, scale=SCALE, bias=ln16_sb[0:HL, :])

    def stage_head(qr, h):
        q0 = qr * QR
        qh = stage.tile([QL, QR], F32R, name="qh")
        nc.sync.dma_start(out=qh, in_=qT[h * QL:(h + 1) * QL, q0:q0 + QR])
        kh = stage.tile([QL, S], F32R, name="kh")
        nc.sync.dma_start(out=kh, in_=kT[h * QL:(h + 1) * QL, 0:S])
        ep = stage.tile([1, QR], F32R, name="ep")
        nc.sync.dma_start(out=ep, in_=e_pad[h:h + 1, qr, :])
        return qh, kh, ep

    def head_loop(qr, h, qh, kh, ep, epi=None):
        """scores/exp/ctx over 16 full k tiles; ctx pipelined two k-tiles
        behind the scores so the exp chain never waits on semaphores; the
        pad block lands as a final K=1 rank-1 update. epi (the previous
        head's epilogue, as a generator) is consumed one q-tile per k-tile
        so its PE transposes hide inside the ACT-bound slack."""
        acc = pp_acc.tile([128, QR], F32, name="acc")
        pend = []  # (et, t) whose ctx matmuls are not yet emitted
        for t in range(NKT - 1):
            if t == 2 and epi is not None:
                for _ in epi:
                    pass
            sc = pp_mm.tile([128, QR], F32, name="mm")
            for j in range(QR // 512):
                nc.tensor.matmul(
                    sc[:, j * 512:(j + 1) * 512],
                    kh[:, t * 128:(t + 1) * 128],
                    qh[:, j * 512:(j + 1) * 512],
                    start=True, stop=True)
            et = expp.tile([128, QR], F32R, name="et")
            nc.scalar.activation(out=et, in_=sc, func=AF.Exp, scale=SCALE)
            pend.append((et, t))
            if len(pend) > 2:
                _emit_ctx(acc, h, *pend.pop(0))
        for p in pend:
            _emit_ctx(acc, h, *p)
        for j in range(QR // 512):
            nc.tensor.matmul(
                acc[0:VL + 1, j * 512:(j + 1) * 512],
                vt[0:1, NKT - 1, h, :],
                ep[0:1, j * 512:(j + 1) * 512],
                start=False, stop=True)
        return acc

    def _emit_ctx(acc, h, et, t):
        for j in range(QR // 512):
            nc.tensor.matmul(
                acc[0:VL + 1, j * 512:(j + 1) * 512],
                vt[0:128, t, h, :],
                et[:, j * 512:(j + 1) * 512],
                start=(t == 0), stop=False)

    def head_epilogue(qr, h, acc, out_sb, dma=False):
        ct = ctp.tile([VL + 1, QR], F32, name="ct")
        nc.vector.tensor_copy(out=ct, in_=acc[0:VL + 1, :])
        for qt in range(QR // 128):
            qsl = slice(qt * 128, (qt + 1) * 128)
            tp = pp_tp.tile([128, 512], F32R, name="tp")
            tpf = tp.bitcast(F32)
            nc.tensor.transpose(
                out=tpf[:, 0:VL + 1],
                in_=ct[:, qsl],
                identity=ident[0:VL + 1, 0:VL + 1])
            rc = rcp.tile([128, 1], F32, name="rc")
            nc.vector.reciprocal(out=rc, in_=tpf[:, VL:VL + 1])
            nc.vector.tensor_scalar_mul(
                out_sb[:, qt, h * VL:(h + 1) * VL], tpf[:, 0:VL], rc)
            if dma:
                r0 = qr * QR + qt * 128
                nc.sync.dma_start(out=y_d[r0:r0 + 128, :], in_=out_sb[:, qt, :])
            yield

    for qr in range(S // QR):               # 2 q ranges of 1024
        out_sb = outp.tile([128, QR // 128, 512], F32, name="out_sb")
        prev = None                          # (h, acc) awaiting epilogue
        for h in range(HL):
            qh, kh, ep = stage_head(qr, h)
            epi = None
            if prev is not None:
                epi = head_epilogue(qr, prev[0], prev[1], out_sb)
            acc = head_loop(qr, h, qh, kh, ep, epi=epi)
            if epi is not None:
                for _ in epi:
                    pass
            prev = (h, acc)
        for _ in head_epilogue(qr, prev[0], prev[1], out_sb, dma=True):
            pass

    for p in (padp, rcp, outp, ctp, expp, stage, pp_acc, pp_tp, pp_mm,
              proj, const):
        p.release()


_NC = None


def _get_nc():
    global _NC
    if _NC is None:
        _NC = _build()
    return _NC


def _shard_inputs(inputs):
    x = np.ascontiguousarray(np.asarray(inputs["x"], dtype=np.float32))
    pad = np.zeros((PAD, D), np.float32)
    ident = np.eye(128, dtype=np.float32)
    bdiag = np.repeat(np.eye(8, dtype=np.float32), 16, axis=0)
    in_maps = []
    for c in range(8):
        b, hg = c // 2, c % 2
        qk = slice(hg * 128, (hg + 1) * 128)
        vv = slice(hg * 512, (hg + 1) * 512)
        in_maps.append({
            "x": np.ascontiguousarray(np.concatenate([x[b], pad], axis=0)),
            "idr": ident,
            "bdiag": bdiag,
            "wq": np.ascontiguousarray(inputs["Wq"][:, qk]),
            "wqc": np.ascontiguousarray(inputs["Wqc"][:, qk]),
            "wk": np.ascontiguousarray(inputs["Wk"][:, qk]),
            "wkc": np.ascontiguousarray(inputs["Wkc"][:, qk]),
            "wv": np.ascontiguousarray(inputs["Wv"][:, vv]),
            "wvc": np.ascontiguousarray(inputs["Wvc"][:, vv]),
            "bq": np.ascontiguousarray(inputs["bq"][qk]),
            "bqc": np.ascontiguousarray(inputs["bqc"][qk]),
            "bk": np.ascontiguousarray(inputs["bk"][qk]),
            "bkc": np.ascontiguousarray(inputs["bkc"][qk]),
            "bv": np.ascontiguousarray(inputs["bv"][vv]),
            "bvc": np.ascontiguousarray(inputs["bvc"][vv]),
        })
    return in_maps


def kernel(**inputs) -> np.ndarray:
    nc = _get_nc()
    in_maps = _shard_inputs(inputs)
    res = run_bass_kernel_spmd(nc, in_maps, list(range(8)))
    out = np.empty((B, S, 1024), np.float32)
    for c in range(8):
        b, hg = c // 2, c % 2
        out[b, :, hg * 512:(hg + 1) * 512] = res.results[c]["y"]
    return out


if __name__ == "__main__":
    rng = np.random.default_rng(0)
    d = 1.0 / np.sqrt(D)
    inputs = {
        "x": rng.standard_normal((B, S, D), dtype=np.float32),
        "Wq": rng.standard_normal((D, 256), dtype=np.float32) * d,
        "bq": rng.standard_normal(256).astype(np.float32) * 0.02,
        "Wqc": rng.standard_normal((D, 256), dtype=np.float32) * d,
        "bqc": rng.standard_normal(256).astype(np.float32) * 0.02,
        "Wk": rng.standard_normal((D, 256), dtype=np.float32) * d,
        "bk": rng.standard_normal(256).astype(np.float32) * 0.02,
        "Wkc": rng.standard_normal((D, 256), dtype=np.float32) * d,
        "bkc": rng.standard_normal(256).astype(np.float32) * 0.02,
        "Wv": rng.standard_normal((D, 1024), dtype=np.float32) * d,
        "bv": rng.standard_normal(1024).astype(np.float32) * 0.02,
        "Wvc": rng.standard_normal((D, 1024), dtype=np.float32) * d,
        "bvc": rng.standard_normal(1024).astype(np.float32) * 0.02,
    }
    y = kernel(**inputs)
    print("kernel output", y.shape, y.dtype, float(np.abs(y).max()))
